# revision 1
# baseline (speedup 1.0000x reference)
"""CQAttention Bass kernel for TRN2, 8 NeuronCores, batch-parallel.

Problem shapes (hardcoded): context [16,128,2048] f32, query [16,128,512] f32,
w [384] f32 -> out [16,512,2048] f32.

Math per batch (D=128, C=2048, Q=512):
  s[c,q]  = bias_c[c] + bias_q[q] + sum_d ctx[d,c]*wcq[d]*qry[d,q]
  s1      = softmax_c(s)            (bias_q is constant along c -> cancels)
  a       = s1 @ qry^T-ish          aT[d,c] = sum_q qryT[q,d] s1T[q,c]
  t[q,d]  = sum_c s1[c,q] ctx[d,c]  (s1^T @ ctx)
  b2T     = sum_q t2[q,d] s1T[q,c]  (associativity: s1@(s1^T@ctx), avoids [C,C])
  out     = [ctxT; aT; ctxT*aT; ctxT*b2T]   ([4D, C] per batch)

Tricks:
  - bias_c folded into the moving operand: s+bias_c = ctx^T @ (qry*wcq + wc)
  - softmax max-subtraction skipped (scores are O(1); exp is safe in fp32)
  - S[q] = sum_c E[c,q] comes free from accum_out on the ET PSUM->SBUF
    copies (free-dim running sum per q-partition)
  - 1/S folded into the tiny per-q-tile lhsT tensors (qryR = qryT/S,
    t2 = t_raw/S^2) so no big tensor is gated on the reciprocal
  - all matmul operands bf16 (full-rate PE; fp32 matmul is 4x slower,
    and fp32r trips walrus's rounding verifier); accumulation is fp32
  - PE warm-up (identity self-transposes) fills the initial DMA-load stall
    and keeps the HAM clock-gate warm for the first real matmuls
"""

import numpy as np

import concourse.bass as bass
import concourse.mybir as mybir
import contextlib as _cl

import concourse.tile as tile
from concourse.bass import ts, ds
from concourse.bass_utils import run_bass_kernel_spmd
from concourse.masks import make_identity

B, D, C, Q = 16, 128, 2048, 512
NCORES = 8
BPC = B // NCORES          # batches per core
NCT = C // 128             # 16 c-tiles
NQT = Q // 128             # 4 q-tiles
NCH = C // 512             # 4 c-chunks
F32 = mybir.dt.float32
BF16 = mybir.dt.bfloat16
AF = mybir.ActivationFunctionType


_SPLIT_TYPES = (
    "InstMatmult", "InstLdweights", "InstActivation", "InstTensorScalar",
    "InstTensorScalarPtr", "InstTensorScalarAffineSelect", "InstTensorTensor",
    "InstTensorCopy", "InstReciprocal", "InstMemset", "InstCopyPredicated",
    "InstBNStats", "InstStreamTranspose", "InstTensorReduce", "InstIota",
    "InstDMACopy", "InstDMA", "InstDMAGather", "InstDMAGatherAnt",
    "InstDrain",
)


def _split_multi_waits(nc, max_embedded=1):
    """walrus allows very few embedded sync-waits per compute instruction
    (AP-parameterized ops seem to have just one slot). Hoist extra waits
    into standalone event-semaphore instructions on the same engine."""
    n = 0
    for fn in nc.m.functions:
        for blk in fn.blocks:
            il = blk.instructions
            i = 0
            while i < len(il):
                inst = il[i]
                si = inst.sync_info
                if (si is not None and si.on_wait
                        and len(si.on_wait) > max_embedded
                        and type(inst).__name__ in _SPLIT_TYPES):
                    waits = list(si.on_wait)
                    extra, keep = waits[:-max_embedded], waits[-max_embedded:]
                    for k, w in enumerate(extra):
                        nop = mybir.InstEventSemaphore(
                            name=f"{inst.name}-w{k}", engine=inst.engine,
                            ins=[], outs=[])
                        nop.sync_info = mybir.SyncInfo(on_wait=[w],
                                                       on_update=[])
                        il.insert(i, nop)
                        i += 1
                        n += 1
                    inst.sync_info = mybir.SyncInfo(on_wait=keep,
                                                    on_update=si.on_update)
                i += 1
    return n


def build_kernel():
    nc = bass.Bass("TRN2", target_bir_lowering=False, debug=False,
                   num_devices=NCORES)
    ctx_ext = nc.dram_tensor("context", [BPC, D, C], F32,
                             kind="ExternalInput").ap()
    qry_ext = nc.dram_tensor("query", [BPC, D, Q], F32,
                             kind="ExternalInput").ap()
    w_ext = nc.dram_tensor("w", [3 * D], F32, kind="ExternalInput").ap()
    out_ext = nc.dram_tensor("out", [BPC, 4 * D, C], F32,
                             kind="ExternalOutput").ap()

    with tile.TileContext(nc) as tc:
        import contextlib
        with contextlib.ExitStack() as ex:
            singles = ex.enter_context(tc.tile_pool(name="singles", bufs=1))
            bb = ex.enter_context(tc.tile_pool(name="bb", bufs=2))
            stg = ex.enter_context(tc.tile_pool(name="stg", bufs=6))
            ps_pool = ex.enter_context(
                tc.tile_pool(name="ps", bufs=2, space="PSUM"))
            tr_pool = ex.enter_context(
                tc.tile_pool(name="tr", bufs=3, space="PSUM"))
            ab_pool = ex.enter_context(
                tc.tile_pool(name="ab", bufs=3, space="PSUM"))

            # ---- constants ----
            ident_bf = singles.tile([128, 128], BF16)
            make_identity(nc, ident_bf)
            # PE warm-up: identity self-transposes during the initial load
            # stall keep the HAM activity window busy so the first real
            # matmuls run at full clock (the copy is the live consumer)
            p_w = tr_pool.tile([128, 1024], BF16, tag="tr")
            for k in range(24):
                nc.tensor.transpose(
                    p_w[:, ts(k % 8, 128)], ident_bf, ident_bf)
            ident_chk = singles.tile([128, 128], BF16)
            nc.vector.tensor_copy(ident_chk, p_w[:, 0:128])
            # wcols[:, 0] = wc, wcols[:, 1] = wcq — one DMA; staged through a
            # DVE copy so consumers carry an engine-sem dep, not a second
            # HWDGE-sem dep (walrus allows only one HWDGE wait per instr).
            wcols_raw = singles.tile([128, 2], F32)
            nc.sync.dma_start(
                out=wcols_raw,
                in_=w_ext[ds(D, 2 * D)].rearrange("(o p) -> p o", o=2))
            wcols = singles.tile([128, 2], F32)
            nc.vector.tensor_copy(wcols, wcols_raw)
            wc_col = wcols[:, 0:1]
            wcq_col = wcols[:, 1:2]

            for b in range(BPC):
                # ---- loads (qry first — it gates the s-matmul rhs chain;
                # ctx chunked so downstream PE work starts early; batch 0's
                # loads get scheduler priority to shrink the startup stall)
                qry_sb = bb.tile([128, Q], F32, tag="qry")
                ctx_sb = bb.tile([128, C], F32, tag="ctx")
                ctx_bf = bb.tile([128, C], BF16, tag="ctxbf")
                prio = tc.high_priority() if b == 0 else _cl.nullcontext()
                with prio:
                    nc.sync.dma_start(out=qry_sb, in_=qry_ext[b])
                    for jl in range(NCH):
                        sl = ts(jl, 512)
                        nc.sync.dma_start(out=ctx_sb[:, sl],
                                          in_=ctx_ext[b][:, sl])
                # qry in bf16 first — it gates qryW2 -> all s-matmuls
                qry_bf = bb.tile([128, Q], BF16, tag="qrybf")
                nc.vector.tensor_copy(qry_bf, qry_sb)
                for jl in range(NCH):
                    sl = ts(jl, 512)
                    # alternate cast engines: gpsimd serializes at ~0.8us per
                    # chunk; DVE is idle this early and takes the odd chunks
                    ceng = nc.vector if jl % 2 else nc.gpsimd
                    ceng.tensor_copy(ctx_bf[:, sl], ctx_sb[:, sl])
                    # section 1: ctx passthrough
                    nc.sync.dma_start(out=out_ext[b, 0:D, sl],
                                      in_=ctx_sb[:, sl])

                # qryW2 = qry*wcq + wc   (bias_c folded into the matmul rhs).
                # Reads the DVE-produced qry_bf so all deps are one DVE sem —
                # ACTIVATE with two AP params has only one wait slot.
                qryW2 = bb.tile([128, Q], BF16, tag="qryW2")
                nc.scalar.activation(qryW2, qry_bf, AF.Identity,
                                     bias=wc_col, scale=wcq_col)
                p_qt = tr_pool.tile([128, 1024], BF16, tag="tr")
                for jq in range(NQT):
                    nc.tensor.transpose(
                        p_qt[:, ts(jq, 128)], qry_bf[:, ts(jq, 128)], ident_bf)
                qryT = bb.tile([128, NQT, 128], BF16, tag="qryT")
                nc.vector.tensor_copy(
                    qryT, p_qt[:, 0:512].rearrange("p (j d) -> p j d", j=NQT))

                # ctxC: ctx transposed to [C-part, D], bf16
                ctxC = bb.tile([128, NCT, 128], BF16, tag="ctxC")
                for jg in range(2):   # groups of 8 c-tiles per bf16 psum buf
                    p_ct = tr_pool.tile([128, 1024], BF16, tag="tr")
                    for jj in range(8):
                        jc = jg * 8 + jj
                        nc.tensor.transpose(
                            p_ct[:, ts(jj, 128)],
                            ctx_bf[:, ts(jc, 128)], ident_bf)
                    nc.vector.tensor_copy(
                        ctxC[:, ds(jg * 8, 8), :],
                        p_ct.rearrange("p (j d) -> p j d", j=8))

                # ---- s-matmuls + exp ----
                E_sb = bb.tile([128, NCT, Q], BF16, tag="E")
                for jc in range(NCT):
                    p_s = ps_pool.tile([128, 512], F32, tag="ps")
                    nc.tensor.matmul(
                        p_s, lhsT=ctx_bf[:, ts(jc, 128)],
                        rhs=qryW2, start=True, stop=True)
                    nc.scalar.activation(E_sb[:, jc, :], p_s, AF.Exp)

                # ---- E transposes -> ET; S[q] via accum_out on copies ----
                ET = bb.tile([128, NQT, C], BF16, tag="ET")
                Spart = bb.tile([128, 2, NQT], F32, tag="Spart")
                # jg-outer: all first-half groups are ready after E tiles
                # 0..7, so psum slot assignment (priority order) matches data
                # readiness and PE never waits on a not-yet-ready group
                for jg in range(2):   # 8 c-tiles per bf16 psum buf
                    for jq in range(NQT):
                        p_et = tr_pool.tile([128, 1024], BF16, tag="tr")
                        for jj in range(8):
                            jc = jg * 8 + jj
                            nc.tensor.transpose(
                                p_et[:, ts(jj, 128)],
                                E_sb[:, jc, ts(jq, 128)], ident_bf)
                        nc.vector.tensor_scalar(
                            out=ET[:, jq, ds(jg * 1024, 1024)], in0=p_et,
                            scalar1=1.0, scalar2=0.0,
                            op0=mybir.AluOpType.mult,
                            op1=mybir.AluOpType.add,
                            accum_out=Spart[:, jg, jq].unsqueeze(1))
                # recip[q] = 1 / sum_c E[c,q]
                Ssum = bb.tile([128, NQT], F32, tag="Ssum")
                nc.vector.tensor_tensor(out=Ssum, in0=Spart[:, 0, :],
                                        in1=Spart[:, 1, :],
                                        op=mybir.AluOpType.add)
                recip = bb.tile([128, NQT], F32, tag="recip")
                nc.vector.reciprocal(recip, Ssum)

                # ---- t_rawT[d,q] = sum_c ctxC[c,d] E[c,q] (one psum bank,
                # shared-lhsT accumulation), then PE-transpose to [q,d] ----
                p_t = ab_pool.tile([128, 512], F32, tag="ab")
                for jc in range(NCT):
                    nc.tensor.matmul(
                        p_t, lhsT=ctxC[:, jc, :], rhs=E_sb[:, jc, :],
                        start=(jc == 0), stop=(jc == NCT - 1))
                tT_bf = bb.tile([128, Q], BF16, tag="tTbf")
                nc.scalar.copy(tT_bf, p_t)
                p_tt = tr_pool.tile([128, 512], BF16, tag="tr")
                for jq in range(NQT):
                    nc.tensor.transpose(
                        p_tt[:, ts(jq, 128)], tT_bf[:, ts(jq, 128)], ident_bf)
                t2 = bb.tile([128, NQT, 128], BF16, tag="t2")
                qryR = bb.tile([128, NQT, 128], BF16, tag="qryR")
                for jq in range(NQT):
                    # t2 = t_raw * recip^2 ; qryR = qryT * recip
                    nc.vector.tensor_scalar(
                        out=t2[:, jq, :], in0=p_tt[:, ts(jq, 128)],
                        scalar1=recip[:, jq:jq + 1],
                        scalar2=recip[:, jq:jq + 1],
                        op0=mybir.AluOpType.mult, op1=mybir.AluOpType.mult)
                    nc.gpsimd.tensor_scalar_mul(
                        qryR[:, jq, :], qryT[:, jq, :], recip[:, jq:jq + 1])

                # ---- a / b2 matmuls + epilogue per c-chunk ----
                for jch in range(NCH):
                    p_a = ab_pool.tile([128, 512], F32, tag="ab")
                    p_b = ab_pool.tile([128, 512], F32, tag="ab")
                    for jq in range(NQT):
                        nc.tensor.matmul(
                            p_a, lhsT=qryR[:, jq, :],
                            rhs=ET[:, jq, ts(jch, 512)],
                            start=(jq == 0), stop=(jq == NQT - 1))
                    for jq in range(NQT):
                        nc.tensor.matmul(
                            p_b, lhsT=t2[:, jq, :],
                            rhs=ET[:, jq, ts(jch, 512)],
                            start=(jq == 0), stop=(jq == NQT - 1))
                    sl = ts(jch, 512)
                    stx = stg.tile([128, 3, 512], F32, tag="stx")
                    if b == 0:
                        nc.vector.tensor_copy(stx[:, 0, :], p_a)
                    else:
                        nc.scalar.copy(stx[:, 0, :], p_a)
                    nc.sync.dma_start(
                        out=out_ext[b, ds(D, D), sl], in_=stx[:, 0, :])
                    nc.vector.tensor_tensor(
                        out=stx[:, 1, :], in0=ctx_sb[:, sl],
                        in1=p_a, op=mybir.AluOpType.mult)
                    nc.sync.dma_start(
                        out=out_ext[b, ds(2 * D, D), sl], in_=stx[:, 1, :])
                    nc.vector.tensor_tensor(
                        out=stx[:, 2, :], in0=ctx_sb[:, sl],
                        in1=p_b, op=mybir.AluOpType.mult)
                    nc.sync.dma_start(
                        out=out_ext[b, ds(3 * D, D), sl], in_=stx[:, 2, :])
    _split_multi_waits(nc)
    return nc


_NC = None


def kernel(context: np.ndarray, query: np.ndarray, w: np.ndarray,
           **extra) -> np.ndarray:
    global _NC
    if _NC is None:
        _NC = build_kernel()
    context = np.ascontiguousarray(context, dtype=np.float32)
    query = np.ascontiguousarray(query, dtype=np.float32)
    w = np.ascontiguousarray(w, dtype=np.float32)
    in_maps = []
    for i in range(NCORES):
        sl = slice(i * BPC, (i + 1) * BPC)
        in_maps.append({
            "context": context[sl],
            "query": query[sl],
            "w": w,
        })
    res = run_bass_kernel_spmd(_NC, in_maps, core_ids=list(range(NCORES)))
    return np.concatenate([r["out"] for r in res.results], axis=0)


if __name__ == "__main__":
    rng = np.random.default_rng(0)
    out = kernel(
        context=rng.standard_normal((B, D, C), dtype=np.float32),
        query=rng.standard_normal((B, D, Q), dtype=np.float32),
        w=(rng.random(3 * D, dtype=np.float32) - 0.5) * 2 / np.sqrt(D),
    )
    print(out.shape, out.dtype)



# revision 3
# speedup vs baseline: 1.1142x; 1.1142x over previous
"""CQAttention Bass kernel for TRN2, 8 NeuronCores, batch-parallel, fp8 PE path.

Problem shapes (hardcoded): context [16,128,2048] f32, query [16,128,512] f32,
w [384] f32 -> out [16,512,2048] f32.

Math per batch (D=128, C=2048, Q=512):
  s[c,q]  = bias_c[c] + bias_q[q] + sum_d ctx[d,c]*wcq[d]*qry[d,q]
  s1      = softmax_c(s)            (bias_q is constant along c -> cancels)
  aT[d,c] = sum_q s1[c,q] qry[d,q]
  t[q,d]  = sum_c s1[c,q] ctx[d,c]
  b2T     = sum_q t2[q,d] s1[c,q]   (assoc: s1(s1^T ctx^T), avoids [C,C])
  out     = [ctxT; aT; ctxT*aT; ctxT*b2T]   ([4D, C] per batch)

Device strategy (all matmuls fp8e4m3 DoubleRow = 0.5 PE-cycles/row):
  - host pre-packs fp8 operand layouts: ctx d-split [64,2,C] (s lhsT),
    qryW2 = qry*wcq+wc d-split [64,2,Q] (s rhs; folds bias_c into the rhs),
    ctxC1 [128,16,129] = ctx^T c-tiled + a trailing ones column, qT [128,4,128]
  - s-matmul -> psum f32 [c-tile, q]; ACT exp(s - 2) -> E fp8 SBUF
    (bias -2 keeps exp below fp8e4m3 max 240; softmax-invariant)
  - ET via PE fp8 transposes: pairs of q-tiles land in the two aligned
    step-2 lanes of one psum region; a single bitcast-uint16 DVE copy
    drains both at the 2x_1p rate (junk odd bytes ride along)
  - t-matmul: lhsT = E c-tile pairs, rhs = ctxC1; the ones column makes
    column 128 of t equal S[q] = sum_c E[c,q]  (softmax sums for free)
  - scale management (fp8 subnormal floor 2^-9): qryR = qT * (2^10/S),
    t2 = t * (2^10/S)^2; sections are descaled in the epilogue against
    host-fed ctx/2^10 and ctx/2^20 bf16 tiles
  - a/b2 DoubleRow over q-tile pairs; rhs = stride-2 fp8 view of ET
  - epilogue per 512-chunk: aT store (DVE tensor_scalar * 2^-10),
    ctx*a / ctx*b2 (tensor_tensor vs pre-scaled ctx; DVE/Pool split)
  - out sections a, ctx*a, ctx*b2 stored bf16; host prepends the exact
    f32 ctx passthrough section and upcasts
"""

import numpy as np
import ml_dtypes

import concourse.bass as bass
import concourse.mybir as mybir
import contextlib as _cl

import concourse.tile as tile
from concourse.bass import ts, ds
from concourse.bass_utils import run_bass_kernel_spmd
from concourse.masks import make_identity

B, D, C, Q = 16, 128, 2048, 512
NCORES = 8
BPC = B // NCORES          # batches per core
NCT = C // 128             # 16 c-tiles
NQT = Q // 128             # 4 q-tiles
NCH = C // 512             # 4 c-chunks
F32 = mybir.dt.float32
BF16 = mybir.dt.bfloat16
F8 = mybir.dt.float8e4
U16 = mybir.dt.uint16
AF = mybir.ActivationFunctionType
DR = mybir.MatmulPerfMode.DoubleRow
MULT = mybir.AluOpType.mult

F8NP = ml_dtypes.float8_e4m3
BFNP = ml_dtypes.bfloat16
SA = 2.0 ** 8              # qryR scale; t2 scale is SA^2
EXPB = -2.0                # exp bias (softmax-invariant)


_SPLIT_TYPES = (
    "InstMatmult", "InstLdweights", "InstActivation", "InstTensorScalar",
    "InstTensorScalarPtr", "InstTensorScalarAffineSelect", "InstTensorTensor",
    "InstTensorCopy", "InstReciprocal", "InstMemset", "InstCopyPredicated",
    "InstBNStats", "InstStreamTranspose", "InstTensorReduce", "InstIota",
    "InstDMACopy", "InstDMA", "InstDMAGather", "InstDMAGatherAnt",
    "InstDrain",
)


def _split_multi_waits(nc, max_embedded=1):
    """walrus allows very few embedded sync-waits per compute instruction
    (AP-parameterized ops seem to have just one slot). Hoist extra waits
    into standalone event-semaphore instructions on the same engine."""
    n = 0
    for fn in nc.m.functions:
        for blk in fn.blocks:
            il = blk.instructions
            i = 0
            while i < len(il):
                inst = il[i]
                si = inst.sync_info
                if (si is not None and si.on_wait
                        and len(si.on_wait) > max_embedded
                        and type(inst).__name__ in _SPLIT_TYPES):
                    waits = list(si.on_wait)
                    extra, keep = waits[:-max_embedded], waits[-max_embedded:]
                    for k, w in enumerate(extra):
                        nop = mybir.InstEventSemaphore(
                            name=f"{inst.name}-w{k}", engine=inst.engine,
                            ins=[], outs=[])
                        nop.sync_info = mybir.SyncInfo(on_wait=[w],
                                                       on_update=[])
                        il.insert(i, nop)
                        i += 1
                        n += 1
                    inst.sync_info = mybir.SyncInfo(on_wait=keep,
                                                    on_update=si.on_update)
                i += 1
    return n


def build_kernel():
    nc = bass.Bass("TRN2", target_bir_lowering=False, debug=False,
                   num_devices=NCORES)
    ctxs_ext = nc.dram_tensor("ctx_s", [BPC, 64, 2, C], F8,
                              kind="ExternalInput").ap()
    qw_ext = nc.dram_tensor("qw", [BPC, 64, 2, Q], F8,
                            kind="ExternalInput").ap()
    ctxc_ext = nc.dram_tensor("ctxc", [BPC, 128, NCT, 132], F8,
                              kind="ExternalInput").ap()
    qt_ext = nc.dram_tensor("qt", [BPC, 128, NQT, 128], F8,
                            kind="ExternalInput").ap()
    ctxa_ext = nc.dram_tensor("ctxa", [BPC, 128, C], BF16,
                              kind="ExternalInput").ap()
    ctxb_ext = nc.dram_tensor("ctxb", [BPC, 128, C], BF16,
                              kind="ExternalInput").ap()
    out_ext = nc.dram_tensor("out", [BPC, 3, 128, C], BF16,
                             kind="ExternalOutput").ap()

    with tile.TileContext(nc) as tc:
        with _cl.ExitStack() as ex:
            singles = ex.enter_context(tc.tile_pool(name="singles", bufs=1))
            bb = ex.enter_context(tc.tile_pool(name="bb", bufs=2))
            stg = ex.enter_context(tc.tile_pool(name="stg", bufs=6))
            ps_s = ex.enter_context(
                tc.tile_pool(name="ps_s", bufs=2, space="PSUM"))
            tr_pool = ex.enter_context(
                tc.tile_pool(name="tr", bufs=2, space="PSUM"))
            ab_pool = ex.enter_context(
                tc.tile_pool(name="ab", bufs=2, space="PSUM"))

            # ---- constants ----
            ident8 = singles.tile([128, 128], F8)
            make_identity(nc, ident8)
            bias_exp = singles.tile([128, 1], F32)
            nc.gpsimd.memset(bias_exp, EXPB)
            # PE warm-up: keeps the PE p-state clock ramped while the first
            # DMA loads land
            p_w = tr_pool.tile([128, 2, 512, 2], F8, tag="tr")
            for k in range(16):
                nc.tensor.transpose(
                    p_w[:, k % 2, ts(k % 4, 128), 0:1], ident8, ident8)
            ident_chk = singles.tile([128, 128], F8)
            nc.vector.tensor_copy(ident_chk, p_w[:, 0, 0:128, 0:1]
                                  .rearrange("p c o -> p (c o)"))

            for b in range(BPC):
                prio = tc.high_priority() if b == 0 else _cl.nullcontext()
                ctxs_sb = bb.tile([64, 2, C], F8, tag="ctxs")
                qw_sb = bb.tile([64, 2, Q], F8, tag="qw")
                ctxc_sb = bb.tile([128, NCT, 132], F8, tag="ctxc")
                qt_sb = bb.tile([128, NQT, 128], F8, tag="qt")
                ctxa_sb = bb.tile([128, C], BF16, tag="ctxa")
                ctxb_sb = bb.tile([128, C], BF16, tag="ctxb")
                with prio:
                    nc.sync.dma_start(out=qw_sb, in_=qw_ext[b])
                    nc.sync.dma_start(out=ctxs_sb, in_=ctxs_ext[b])
                    nc.sync.dma_start(out=ctxc_sb, in_=ctxc_ext[b])
                    nc.sync.dma_start(out=qt_sb, in_=qt_ext[b])
                nc.sync.dma_start(out=ctxa_sb, in_=ctxa_ext[b])
                nc.sync.dma_start(out=ctxb_sb, in_=ctxb_ext[b])

                # ---- s-matmuls + exp: 8 groups of 2 c-tiles ----
                E8 = bb.tile([128, NCT, Q], F8, tag="E")
                for g in range(8):
                    p_s = ps_s.tile([128, 2, Q], F32, tag="s")
                    for jj in range(2):
                        nc.tensor.matmul(
                            p_s[:, jj, :],
                            lhsT=ctxs_sb[:, :, ts(2 * g + jj, 128)],
                            rhs=qw_sb, start=True, stop=True, perf_mode=DR)
                    nc.scalar.activation(
                        E8[:, ds(2 * g, 2), :], p_s, AF.Exp,
                        bias=bias_exp, scale=1.0)

                # ---- ET via paired fp8 transposes + uint16 copies ----
                # ET[q, jqp, jch, kt, c, lane]: value lane 0, junk lane 1
                ET = bb.tile([128, 2, NCH, 2, 512, 2], F8, tag="ET")
                for jqp in range(2):
                    for jch in range(NCH):
                        p_tr = tr_pool.tile([128, 2, 512, 2], F8, tag="tr")
                        for kt in range(2):
                            jq = 2 * jqp + kt
                            for j4 in range(4):
                                jc = 4 * jch + j4
                                nc.tensor.transpose(
                                    p_tr[:, kt, ds(128 * j4, 128), 0:1],
                                    E8[:, jc, ts(jq, 128)], ident8)
                        nc.vector.tensor_copy(
                            ET[:, jqp, jch].bitcast(U16), p_tr.bitcast(U16))

                # ---- t-matmul (ones column -> S) + scales ----
                t2 = bb.tile([128, NQT, 128], F8, tag="t2")
                qryR = bb.tile([128, NQT, 128], F8, tag="qryR")
                recipA = bb.tile([128, NQT], F32, tag="recipA")
                for jq in range(NQT):
                    p_t = ab_pool.tile([128, 512], F32, tag="ab")
                    for i in range(8):
                        nc.tensor.matmul(
                            p_t[:, 0:129],
                            lhsT=E8[:, ds(2 * i, 2), ts(jq, 128)],
                            rhs=ctxc_sb[:, ds(2 * i, 2), 0:129],
                            start=(i == 0), stop=(i == 7), perf_mode=DR)
                    # ones column holds 1/SA, so S-col = S/SA and
                    # reciprocal directly yields recipA = SA / S
                    nc.vector.reciprocal(recipA[:, jq:jq + 1],
                                         p_t[:, 128:129])
                    nc.gpsimd.tensor_scalar_mul(
                        qryR[:, jq, :], qt_sb[:, jq, :],
                        recipA[:, jq:jq + 1])
                    nc.vector.tensor_scalar(
                        out=t2[:, jq, :], in0=p_t[:, 0:128],
                        scalar1=recipA[:, jq:jq + 1],
                        scalar2=recipA[:, jq:jq + 1],
                        op0=MULT, op1=MULT)

                # ---- a/b2 matmuls + epilogue per c-chunk ----
                for jch in range(NCH):
                    p_a = ab_pool.tile([128, 512], F32, tag="ab")
                    p_b = ab_pool.tile([128, 512], F32, tag="ab")
                    for jqp in range(2):
                        rhs = ET[:, jqp, jch, :, :, 0:1]
                        nc.tensor.matmul(
                            p_a, lhsT=qryR[:, ds(2 * jqp, 2), :], rhs=rhs,
                            start=(jqp == 0), stop=(jqp == 1), perf_mode=DR)
                    for jqp in range(2):
                        rhs = ET[:, jqp, jch, :, :, 0:1]
                        nc.tensor.matmul(
                            p_b, lhsT=t2[:, ds(2 * jqp, 2), :], rhs=rhs,
                            start=(jqp == 0), stop=(jqp == 1), perf_mode=DR)
                    sl = ts(jch, 512)
                    stx = stg.tile([128, 3, 512], BF16, tag="stx")
                    # section a: descale by 2^-10
                    nc.vector.tensor_scalar_mul(stx[:, 0, :], p_a, 1.0 / SA)
                    nc.sync.dma_start(out=out_ext[b, 0, :, sl],
                                      in_=stx[:, 0, :])
                    # section ctx*a: ctxa = ctx/2^10
                    eng = nc.vector if jch % 2 else nc.gpsimd
                    eng.tensor_tensor(
                        out=stx[:, 1, :], in0=ctxa_sb[:, sl], in1=p_a,
                        op=MULT)
                    nc.sync.dma_start(out=out_ext[b, 1, :, sl],
                                      in_=stx[:, 1, :])
                    # section ctx*b2: ctxb = ctx/2^20
                    eng2 = nc.gpsimd if jch % 2 else nc.vector
                    eng2.tensor_tensor(
                        out=stx[:, 2, :], in0=ctxb_sb[:, sl], in1=p_b,
                        op=MULT)
                    nc.sync.dma_start(out=out_ext[b, 2, :, sl],
                                      in_=stx[:, 2, :])
    _split_multi_waits(nc)
    return nc


_NC = None


def _prep_inputs(context, query, w):
    """Host-side sharding + fp8/bf16 operand packing."""
    f32 = np.float32
    context = np.ascontiguousarray(context, dtype=f32)
    query = np.ascontiguousarray(query, dtype=f32)
    w = np.ascontiguousarray(w, dtype=f32)
    wc, wcq = w[D:2 * D], w[2 * D:3 * D]

    ctx_s = np.ascontiguousarray(
        context.reshape(B, 2, 64, C).transpose(0, 2, 1, 3)).astype(F8NP)
    qw = np.ascontiguousarray(
        (query * wcq[None, :, None] + wc[None, :, None])
        .reshape(B, 2, 64, Q).transpose(0, 2, 1, 3)).astype(F8NP)
    # ctxc[b, p, j, 0:128] = ctx[b, :, 128j+p]; col 128 = 1.0 (S column)
    ctxc = np.zeros((B, 128, NCT, 132), dtype=F8NP)
    ctxc[:, :, :, 0:128] = (context.transpose(0, 2, 1)
                            .reshape(B, NCT, 128, D)
                            .transpose(0, 2, 1, 3)).astype(F8NP)
    ctxc[:, :, :, 128] = np.float32(1.0 / SA).astype(F8NP)
    qt = np.ascontiguousarray(
        query.transpose(0, 2, 1).reshape(B, NQT, 128, D)
        .transpose(0, 2, 1, 3)).astype(F8NP)
    ctxa = (context * (1.0 / SA)).astype(BFNP)
    ctxb = (context * (1.0 / SA ** 2)).astype(BFNP)

    in_maps = []
    for i in range(NCORES):
        sl = slice(i * BPC, (i + 1) * BPC)
        in_maps.append({
            "ctx_s": ctx_s[sl], "qw": qw[sl], "ctxc": ctxc[sl],
            "qt": qt[sl], "ctxa": ctxa[sl], "ctxb": ctxb[sl],
        })
    return context, in_maps


def kernel(context: np.ndarray, query: np.ndarray, w: np.ndarray,
           **extra) -> np.ndarray:
    global _NC
    if _NC is None:
        _NC = build_kernel()
    context, in_maps = _prep_inputs(context, query, w)
    res = run_bass_kernel_spmd(_NC, in_maps, core_ids=list(range(NCORES)))
    dev = np.concatenate([r["out"] for r in res.results], axis=0)  # [B,3,128,C]
    out = np.empty((B, 4 * D, C), dtype=np.float32)
    out[:, 0:D, :] = context
    out[:, D:4 * D, :] = dev.astype(np.float32).reshape(B, 3 * D, C)
    return out


if __name__ == "__main__":
    rng = np.random.default_rng(0)
    out = kernel(
        context=rng.standard_normal((B, D, C), dtype=np.float32),
        query=rng.standard_normal((B, D, Q), dtype=np.float32),
        w=(rng.random(3 * D, dtype=np.float32) - 0.5) * 2 / np.sqrt(D),
    )
    print(out.shape, out.dtype)


# revision 5
# speedup vs baseline: 1.1401x; 1.0232x over previous
"""CQAttention Bass kernel for TRN2, 8 NeuronCores, batch-parallel, fp8 PE path.

Problem shapes (hardcoded): context [16,128,2048] f32, query [16,128,512] f32,
w [384] f32 -> out [16,512,2048] f32.

Math per batch (D=128, C=2048, Q=512):
  s[c,q]  = bias_c[c] + bias_q[q] + sum_d ctx[d,c]*wcq[d]*qry[d,q]
  s1      = softmax_c(s)            (bias_q is constant along c -> cancels)
  aT[d,c] = sum_q s1[c,q] qry[d,q]
  t[q,d]  = sum_c s1[c,q] ctx[d,c]
  b2T     = sum_q t2[q,d] s1[c,q]   (assoc: s1(s1^T ctx^T), avoids [C,C])
  out     = [ctxT; aT; ctxT*aT; ctxT*b2T]   ([4D, C] per batch)

Device strategy (all matmuls fp8e4m3 DoubleRow = 0.5 PE-cycles/row):
  - host pre-packs fp8 operand layouts: ctx d-split [64,2,C] (s lhsT),
    qryW2 = qry*wcq+wc d-split [64,2,Q] (s rhs; folds bias_c into the rhs),
    ctxC1 [128,16,129] = ctx^T c-tiled + a trailing ones column, qT [128,4,128]
  - s-matmul -> psum f32 [c-tile, q]; ACT exp(s - 2) -> E fp8 SBUF
    (bias -2 keeps exp below fp8e4m3 max 240; softmax-invariant)
  - ET via PE fp8 transposes: pairs of q-tiles land in the two aligned
    step-2 lanes of one psum region; a single bitcast-uint16 DVE copy
    drains both at the 2x_1p rate (junk odd bytes ride along)
  - t-matmul: lhsT = E c-tile pairs, rhs = ctxC1; the ones column makes
    column 128 of t equal S[q] = sum_c E[c,q]  (softmax sums for free)
  - scale management (fp8 subnormal floor 2^-9): qryR = qT * (2^10/S),
    t2 = t * (2^10/S)^2; sections are descaled in the epilogue against
    host-fed ctx/2^10 and ctx/2^20 bf16 tiles
  - a/b2 DoubleRow over q-tile pairs; rhs = stride-2 fp8 view of ET
  - epilogue per 512-chunk: aT store (DVE tensor_scalar * 2^-10),
    ctx*a / ctx*b2 (tensor_tensor vs pre-scaled ctx; DVE/Pool split)
  - out sections a, ctx*a, ctx*b2 stored bf16; host prepends the exact
    f32 ctx passthrough section and upcasts
"""

import numpy as np
import ml_dtypes

import concourse.bass as bass
import concourse.mybir as mybir
import contextlib as _cl

import concourse.tile as tile
from concourse.bass import ts, ds
from concourse.bass_utils import run_bass_kernel_spmd
from concourse.masks import make_identity

B, D, C, Q = 16, 128, 2048, 512
NCORES = 8
BPC = B // NCORES          # batches per core
NCT = C // 128             # 16 c-tiles
NQT = Q // 128             # 4 q-tiles
NCH = C // 512             # 4 c-chunks
F32 = mybir.dt.float32
BF16 = mybir.dt.bfloat16
F8 = mybir.dt.float8e4
U16 = mybir.dt.uint16
AF = mybir.ActivationFunctionType
DR = mybir.MatmulPerfMode.DoubleRow
MULT = mybir.AluOpType.mult

F8NP = ml_dtypes.float8_e4m3
BFNP = ml_dtypes.bfloat16
SA = 2.0 ** 8              # qryR scale; t2 scale is SA^2
EXPB = -2.0                # exp bias (softmax-invariant)


_SPLIT_TYPES = (
    "InstMatmult", "InstLdweights", "InstActivation", "InstTensorScalar",
    "InstTensorScalarPtr", "InstTensorScalarAffineSelect", "InstTensorTensor",
    "InstTensorCopy", "InstReciprocal", "InstMemset", "InstCopyPredicated",
    "InstBNStats", "InstStreamTranspose", "InstTensorReduce", "InstIota",
    "InstDMACopy", "InstDMA", "InstDMAGather", "InstDMAGatherAnt",
    "InstDrain",
)


def _split_multi_waits(nc, max_embedded=1):
    """walrus allows very few embedded sync-waits per compute instruction
    (AP-parameterized ops seem to have just one slot). Hoist extra waits
    into standalone event-semaphore instructions on the same engine."""
    n = 0
    for fn in nc.m.functions:
        for blk in fn.blocks:
            il = blk.instructions
            i = 0
            while i < len(il):
                inst = il[i]
                si = inst.sync_info
                if (si is not None and si.on_wait
                        and len(si.on_wait) > max_embedded
                        and type(inst).__name__ in _SPLIT_TYPES):
                    waits = list(si.on_wait)
                    extra, keep = waits[:-max_embedded], waits[-max_embedded:]
                    for k, w in enumerate(extra):
                        nop = mybir.InstEventSemaphore(
                            name=f"{inst.name}-w{k}", engine=inst.engine,
                            ins=[], outs=[])
                        nop.sync_info = mybir.SyncInfo(on_wait=[w],
                                                       on_update=[])
                        il.insert(i, nop)
                        i += 1
                        n += 1
                    inst.sync_info = mybir.SyncInfo(on_wait=keep,
                                                    on_update=si.on_update)
                i += 1
    return n


def build_kernel():
    nc = bass.Bass("TRN2", target_bir_lowering=False, debug=False,
                   num_devices=NCORES)
    # packed inputs: one 64-partition and one 128-partition byte tensor
    # pk64[b]  = ctx_s [64,2,C] fp8 (4096B) ++ qw [64,2,Q] fp8 (1024B)
    # pk128[b] = ctxc [128,16,132] fp8 (2112B) ++ qt [128,4,128] fp8 (512B)
    #            ++ ctx8 [128,C] fp8 (2048B)
    U8 = mybir.dt.uint8
    pk64_ext = nc.dram_tensor("pk64", [BPC, 64, 5120], U8,
                              kind="ExternalInput").ap()
    pk128_ext = nc.dram_tensor("pk128", [BPC, 128, 4672], U8,
                               kind="ExternalInput").ap()
    out_ext = nc.dram_tensor("out", [BPC, 3, 128, C], BF16,
                             kind="ExternalOutput").ap()

    with tile.TileContext(nc) as tc:
        with _cl.ExitStack() as ex:
            singles = ex.enter_context(tc.tile_pool(name="singles", bufs=1))
            bb = ex.enter_context(tc.tile_pool(name="bb", bufs=2))
            stg = ex.enter_context(tc.tile_pool(name="stg", bufs=6))
            ps_s = ex.enter_context(
                tc.tile_pool(name="ps_s", bufs=2, space="PSUM"))
            tr_pool = ex.enter_context(
                tc.tile_pool(name="tr", bufs=2, space="PSUM"))
            ab_pool = ex.enter_context(
                tc.tile_pool(name="ab", bufs=2, space="PSUM"))

            # ---- constants ----
            ident8 = singles.tile([128, 128], F8)
            make_identity(nc, ident8)
            bias_exp = singles.tile([128, 1], F32)
            nc.gpsimd.memset(bias_exp, EXPB)
            # PE warm-up: keeps the PE p-state clock ramped while the first
            # DMA loads land
            p_w = tr_pool.tile([128, 2, 512, 2], F8, tag="tr")
            for k in range(16):
                nc.tensor.transpose(
                    p_w[:, k % 2, ts(k % 4, 128), 0:1], ident8, ident8)
            ident_chk = singles.tile([128, 128], F8)
            nc.vector.tensor_copy(ident_chk, p_w[:, 0, 0:128, 0:1]
                                  .rearrange("p c o -> p (c o)"))

            U8 = mybir.dt.uint8
            for b in range(BPC):
                prio = tc.high_priority() if b == 0 else _cl.nullcontext()
                pk64_sb = bb.tile([64, 5120], U8, tag="pk64")
                pk128_sb = bb.tile([128, 4672], U8, tag="pk128")
                with prio:
                    nc.sync.dma_start(out=pk64_sb, in_=pk64_ext[b])
                    nc.sync.dma_start(out=pk128_sb, in_=pk128_ext[b])
                ctxs_sb = pk64_sb[:, 0:4096].bitcast(F8).rearrange(
                    "p (h c) -> p h c", h=2)
                qw_sb = pk64_sb[:, 4096:5120].bitcast(F8).rearrange(
                    "p (h q) -> p h q", h=2)
                ctxc_sb = pk128_sb[:, 0:2112].bitcast(F8).rearrange(
                    "p (j w) -> p j w", j=NCT)
                qt_sb = pk128_sb[:, 2112:2624].bitcast(F8).rearrange(
                    "p (j d) -> p j d", j=NQT)
                ctx8_sb = pk128_sb[:, 2624:4672].bitcast(F8)

                # ---- s-matmuls + exp: 8 groups of 2 c-tiles ----
                E8 = bb.tile([128, NCT, Q], F8, tag="E")
                for g in range(8):
                    p_s = ps_s.tile([128, 2, Q], F32, tag="s")
                    for jj in range(2):
                        nc.tensor.matmul(
                            p_s[:, jj, :],
                            lhsT=ctxs_sb[:, :, ts(2 * g + jj, 128)],
                            rhs=qw_sb, start=True, stop=True, perf_mode=DR)
                    nc.scalar.activation(
                        E8[:, ds(2 * g, 2), :], p_s, AF.Exp,
                        bias=bias_exp, scale=1.0)

                # ---- ET via paired fp8 transposes + uint16 copies ----
                # ET[q, jqp, jch, kt, c, lane]: value lane 0, junk lane 1
                ET = bb.tile([128, 2, NCH, 2, 512, 2], F8, tag="ET")
                for jqp in range(2):
                    for jch in range(NCH):
                        p_tr = tr_pool.tile([128, 2, 512, 2], F8, tag="tr")
                        for kt in range(2):
                            jq = 2 * jqp + kt
                            for j4 in range(4):
                                jc = 4 * jch + j4
                                nc.tensor.transpose(
                                    p_tr[:, kt, ds(128 * j4, 128), 0:1],
                                    E8[:, jc, ts(jq, 128)], ident8)
                        nc.vector.tensor_copy(
                            ET[:, jqp, jch].bitcast(U16), p_tr.bitcast(U16))

                # ---- t-matmul (ones column -> S) + scales ----
                t2 = bb.tile([128, NQT, 128], F8, tag="t2")
                qryR = bb.tile([128, NQT, 128], F8, tag="qryR")
                recipA = bb.tile([128, NQT], F32, tag="recipA")
                for jq in range(NQT):
                    p_t = ab_pool.tile([128, 512], F32, tag="ab")
                    for i in range(8):
                        nc.tensor.matmul(
                            p_t[:, 0:129],
                            lhsT=E8[:, ds(2 * i, 2), ts(jq, 128)],
                            rhs=ctxc_sb[:, ds(2 * i, 2), 0:129],
                            start=(i == 0), stop=(i == 7), perf_mode=DR)
                    # ones column holds 1/SA, so S-col = S/SA and
                    # reciprocal directly yields recipA = SA / S
                    nc.vector.reciprocal(recipA[:, jq:jq + 1],
                                         p_t[:, 128:129])
                    nc.gpsimd.tensor_scalar_mul(
                        qryR[:, jq, :], qt_sb[:, jq, :],
                        recipA[:, jq:jq + 1])
                    nc.vector.tensor_scalar(
                        out=t2[:, jq, :], in0=p_t[:, 0:128],
                        scalar1=recipA[:, jq:jq + 1],
                        scalar2=recipA[:, jq:jq + 1],
                        op0=MULT, op1=MULT)

                # ---- a/b2 matmuls + epilogue per c-chunk ----
                sec_a = stg.tile([128, C], BF16, tag="sec_a")
                sec_ca = stg.tile([128, C], BF16, tag="sec_ca")
                sec_cb = stg.tile([128, C], BF16, tag="sec_cb")
                for jch in range(NCH):
                    p_a = ab_pool.tile([128, 512], F32, tag="ab")
                    p_b = ab_pool.tile([128, 512], F32, tag="ab")
                    for jqp in range(2):
                        rhs = ET[:, jqp, jch, :, :, 0:1]
                        nc.tensor.matmul(
                            p_a, lhsT=qryR[:, ds(2 * jqp, 2), :], rhs=rhs,
                            start=(jqp == 0), stop=(jqp == 1), perf_mode=DR)
                    for jqp in range(2):
                        rhs = ET[:, jqp, jch, :, :, 0:1]
                        nc.tensor.matmul(
                            p_b, lhsT=t2[:, ds(2 * jqp, 2), :], rhs=rhs,
                            start=(jqp == 0), stop=(jqp == 1), perf_mode=DR)
                    sl = ts(jch, 512)
                    # section a: descale by 1/SA (ACT helps on the tail batch)
                    if b == BPC - 1:
                        nc.scalar.mul(sec_a[:, sl], p_a, 1.0 / SA)
                    else:
                        nc.vector.tensor_scalar_mul(sec_a[:, sl], p_a, 1.0 / SA)
                    # section ctx*a = (p_a / SA) * ctx   (fused descale)
                    nc.vector.scalar_tensor_tensor(
                        out=sec_ca[:, sl], in0=p_a, scalar=1.0 / SA,
                        in1=ctx8_sb[:, sl], op0=MULT, op1=MULT)
                    # section ctx*b2 = (p_b / SA^2) * ctx
                    nc.gpsimd.scalar_tensor_tensor(
                        out=sec_cb[:, sl], in0=p_b, scalar=1.0 / SA ** 2,
                        in1=ctx8_sb[:, sl], op0=MULT, op1=MULT)
                nc.sync.dma_start(out=out_ext[b, 0], in_=sec_a)
                nc.sync.dma_start(out=out_ext[b, 1], in_=sec_ca)
                nc.sync.dma_start(out=out_ext[b, 2], in_=sec_cb)
    _split_multi_waits(nc)
    return nc


_NC = None


def _prep_inputs(context, query, w):
    """Host-side sharding + fp8 packing into the two byte tensors."""
    f32 = np.float32
    context = np.ascontiguousarray(context, dtype=f32)
    query = np.ascontiguousarray(query, dtype=f32)
    w = np.ascontiguousarray(w, dtype=f32)
    wc, wcq = w[D:2 * D], w[2 * D:3 * D]

    ctx_s = np.ascontiguousarray(
        context.reshape(B, 2, 64, C).transpose(0, 2, 1, 3)).astype(F8NP)
    qw = np.ascontiguousarray(
        (query * wcq[None, :, None] + wc[None, :, None])
        .reshape(B, 2, 64, Q).transpose(0, 2, 1, 3)).astype(F8NP)
    pk64 = np.concatenate([
        ctx_s.reshape(B, 64, 2 * C).view(np.uint8),
        qw.reshape(B, 64, 2 * Q).view(np.uint8)], axis=2)

    # ctxc[b, p, j, 0:128] = ctx[b, :, 128j+p]; col 128 = 1/SA (S column)
    ctxc = np.zeros((B, 128, NCT, 132), dtype=F8NP)
    ctxc[:, :, :, 0:128] = (context.transpose(0, 2, 1)
                            .reshape(B, NCT, 128, D)
                            .transpose(0, 2, 1, 3)).astype(F8NP)
    ctxc[:, :, :, 128] = np.float32(1.0 / SA).astype(F8NP)
    qt = np.ascontiguousarray(
        query.transpose(0, 2, 1).reshape(B, NQT, 128, D)
        .transpose(0, 2, 1, 3)).astype(F8NP)
    ctx8 = context.astype(F8NP)
    pk128 = np.concatenate([
        ctxc.reshape(B, 128, NCT * 132).view(np.uint8),
        qt.reshape(B, 128, NQT * 128).view(np.uint8),
        ctx8.view(np.uint8)], axis=2)

    in_maps = []
    for i in range(NCORES):
        sl = slice(i * BPC, (i + 1) * BPC)
        in_maps.append({"pk64": pk64[sl], "pk128": pk128[sl]})
    return context, in_maps


def kernel(context: np.ndarray, query: np.ndarray, w: np.ndarray,
           **extra) -> np.ndarray:
    global _NC
    if _NC is None:
        _NC = build_kernel()
    context, in_maps = _prep_inputs(context, query, w)
    res = run_bass_kernel_spmd(_NC, in_maps, core_ids=list(range(NCORES)))
    dev = np.concatenate([r["out"] for r in res.results], axis=0)  # [B,3,128,C]
    out = np.empty((B, 4 * D, C), dtype=np.float32)
    out[:, 0:D, :] = context
    out[:, D:4 * D, :] = dev.astype(np.float32).reshape(B, 3 * D, C)
    return out


if __name__ == "__main__":
    rng = np.random.default_rng(0)
    out = kernel(
        context=rng.standard_normal((B, D, C), dtype=np.float32),
        query=rng.standard_normal((B, D, Q), dtype=np.float32),
        w=(rng.random(3 * D, dtype=np.float32) - 0.5) * 2 / np.sqrt(D),
    )
    print(out.shape, out.dtype)


# revision 6
# speedup vs baseline: 1.2332x; 1.0817x over previous
"""CQAttention Bass kernel for TRN2, 8 NeuronCores, batch-parallel, fp8 PE path.

Problem shapes (hardcoded): context [16,128,2048] f32, query [16,128,512] f32,
w [384] f32 -> out [16,512,2048] f32.

Math per batch (D=128, C=2048, Q=512):
  s[c,q]  = bias_c[c] + bias_q[q] + sum_d ctx[d,c]*wcq[d]*qry[d,q]
  s1      = softmax_c(s)            (bias_q is constant along c -> cancels)
  aT[d,c] = sum_q s1[c,q] qry[d,q]
  t[q,d]  = sum_c s1[c,q] ctx[d,c]
  b2T     = sum_q t2[q,d] s1[c,q]   (assoc: s1(s1^T ctx^T), avoids [C,C])
  out     = [ctxT; aT; ctxT*aT; ctxT*b2T]   ([4D, C] per batch)

Device strategy (all matmuls fp8e4m3 DoubleRow = 0.5 PE-cycles/row):
  - host pre-packs fp8 operand layouts: ctx d-split [64,2,C] (s lhsT),
    qryW2 = qry*wcq+wc d-split [64,2,Q] (s rhs; folds bias_c into the rhs),
    ctxC1 [128,16,129] = ctx^T c-tiled + a trailing ones column, qT [128,4,128]
  - s-matmul -> psum f32 [c-tile, q]; ACT exp(s - 2) -> E fp8 SBUF
    (bias -2 keeps exp below fp8e4m3 max 240; softmax-invariant)
  - ET via PE fp8 transposes: pairs of q-tiles land in the two aligned
    step-2 lanes of one psum region; a single bitcast-uint16 DVE copy
    drains both at the 2x_1p rate (junk odd bytes ride along)
  - t-matmul: lhsT = E c-tile pairs, rhs = ctxC1; the ones column makes
    column 128 of t equal S[q] = sum_c E[c,q]  (softmax sums for free)
  - scale management (fp8 subnormal floor 2^-9): qryR = qT * (2^10/S),
    t2 = t * (2^10/S)^2; sections are descaled in the epilogue against
    host-fed ctx/2^10 and ctx/2^20 bf16 tiles
  - a/b2 DoubleRow over q-tile pairs; rhs = stride-2 fp8 view of ET
  - epilogue per 512-chunk: aT store (DVE tensor_scalar * 2^-10),
    ctx*a / ctx*b2 (tensor_tensor vs pre-scaled ctx; DVE/Pool split)
  - out sections a, ctx*a, ctx*b2 stored bf16; host prepends the exact
    f32 ctx passthrough section and upcasts
"""

import numpy as np
import ml_dtypes

import concourse.bass as bass
import concourse.mybir as mybir
import contextlib as _cl

import concourse.tile as tile
from concourse.bass import ts, ds
from concourse.bass_utils import run_bass_kernel_spmd
from concourse.masks import make_identity

B, D, C, Q = 16, 128, 2048, 512
NCORES = 8
BPC = B // NCORES          # batches per core
NCT = C // 128             # 16 c-tiles
NQT = Q // 128             # 4 q-tiles
NCH = C // 512             # 4 c-chunks
F32 = mybir.dt.float32
BF16 = mybir.dt.bfloat16
F8 = mybir.dt.float8e4
U16 = mybir.dt.uint16
AF = mybir.ActivationFunctionType
DR = mybir.MatmulPerfMode.DoubleRow
MULT = mybir.AluOpType.mult

F8NP = ml_dtypes.float8_e4m3
BFNP = ml_dtypes.bfloat16
SA = 2.0 ** 8              # qryR scale; t2 scale is SA^2
EXPB = -2.0                # exp bias (softmax-invariant)


_SPLIT_TYPES = (
    "InstMatmult", "InstLdweights", "InstActivation", "InstTensorScalar",
    "InstTensorScalarPtr", "InstTensorScalarAffineSelect", "InstTensorTensor",
    "InstTensorCopy", "InstReciprocal", "InstMemset", "InstCopyPredicated",
    "InstBNStats", "InstStreamTranspose", "InstTensorReduce", "InstIota",
    "InstDMACopy", "InstDMA", "InstDMAGather", "InstDMAGatherAnt",
    "InstDrain",
)


def _split_multi_waits(nc, max_embedded=1):
    """walrus allows very few embedded sync-waits per compute instruction
    (AP-parameterized ops seem to have just one slot). Hoist extra waits
    into standalone event-semaphore instructions on the same engine."""
    n = 0
    for fn in nc.m.functions:
        for blk in fn.blocks:
            il = blk.instructions
            i = 0
            while i < len(il):
                inst = il[i]
                si = inst.sync_info
                if (si is not None and si.on_wait
                        and len(si.on_wait) > max_embedded
                        and type(inst).__name__ in _SPLIT_TYPES):
                    waits = list(si.on_wait)
                    extra, keep = waits[:-max_embedded], waits[-max_embedded:]
                    for k, w in enumerate(extra):
                        nop = mybir.InstEventSemaphore(
                            name=f"{inst.name}-w{k}", engine=inst.engine,
                            ins=[], outs=[])
                        nop.sync_info = mybir.SyncInfo(on_wait=[w],
                                                       on_update=[])
                        il.insert(i, nop)
                        i += 1
                        n += 1
                    inst.sync_info = mybir.SyncInfo(on_wait=keep,
                                                    on_update=si.on_update)
                i += 1
    return n


def build_kernel():
    nc = bass.Bass("TRN2", target_bir_lowering=False, debug=False,
                   num_devices=NCORES)
    # packed inputs: one 64-partition and one 128-partition byte tensor
    # pk64[b]  = ctx_s [64,2,C] fp8 (4096B) ++ qw [64,2,Q] fp8 (1024B)
    # pk128[b] = ctxc [128,16,132] fp8 (2112B) ++ qt [128,4,128] fp8 (512B)
    #            ++ ctx8 [128,C] fp8 (2048B)
    U8 = mybir.dt.uint8
    pk64_ext = nc.dram_tensor("pk64", [BPC, 64, 5120], U8,
                              kind="ExternalInput").ap()
    pk128_ext = nc.dram_tensor("pk128", [BPC, 128, 4672], U8,
                               kind="ExternalInput").ap()
    out_ext = nc.dram_tensor("out", [BPC, 3, 128, C], BF16,
                             kind="ExternalOutput").ap()

    with tile.TileContext(nc) as tc:
        with _cl.ExitStack() as ex:
            singles = ex.enter_context(tc.tile_pool(name="singles", bufs=1))
            bb = ex.enter_context(tc.tile_pool(name="bb", bufs=2))
            stg = ex.enter_context(tc.tile_pool(name="stg", bufs=6))
            ps_s = ex.enter_context(
                tc.tile_pool(name="ps_s", bufs=2, space="PSUM"))
            tr_pool = ex.enter_context(
                tc.tile_pool(name="tr", bufs=2, space="PSUM"))
            ab_pool = ex.enter_context(
                tc.tile_pool(name="ab", bufs=2, space="PSUM"))

            # ---- constants ----
            ident8 = singles.tile([128, 128], F8)
            make_identity(nc, ident8)
            bias_exp = singles.tile([128, 1], F32)
            nc.gpsimd.memset(bias_exp, EXPB)
            # PE warm-up: keeps the PE p-state clock ramped while the first
            # DMA loads land
            p_w = tr_pool.tile([128, 2, 512, 2], F8, tag="tr")
            for k in range(16):
                nc.tensor.transpose(
                    p_w[:, k % 2, ts(k % 4, 128), 0:1], ident8, ident8)
            ident_chk = singles.tile([128, 128], F8)
            nc.vector.tensor_copy(ident_chk, p_w[:, 0, 0:128, 0:1]
                                  .rearrange("p c o -> p (c o)"))

            U8 = mybir.dt.uint8
            for b in range(BPC):
                prio = tc.high_priority() if b == 0 else _cl.nullcontext()
                pk64_sb = bb.tile([64, 5120], U8, tag="pk64")
                pk128_sb = bb.tile([128, 4672], U8, tag="pk128")
                with prio:
                    nc.sync.dma_start(out=pk64_sb, in_=pk64_ext[b])
                    nc.sync.dma_start(out=pk128_sb, in_=pk128_ext[b])
                ctxs_sb = pk64_sb[:, 0:4096].bitcast(F8).rearrange(
                    "p (h c) -> p h c", h=2)
                qw_sb = pk64_sb[:, 4096:5120].bitcast(F8).rearrange(
                    "p (h q) -> p h q", h=2)
                ctxc_sb = pk128_sb[:, 0:2112].bitcast(F8).rearrange(
                    "p (j w) -> p j w", j=NCT)
                qt_sb = pk128_sb[:, 2112:2624].bitcast(F8).rearrange(
                    "p (j d) -> p j d", j=NQT)
                ctx8_sb = pk128_sb[:, 2624:4672].bitcast(F8)

                # ---- s-matmuls + exp: 8 groups of 2 c-tiles ----
                E8 = bb.tile([128, NCT, Q], F8, tag="E")
                for g in range(8):
                    p_s = ps_s.tile([128, 2, Q], F32, tag="s")
                    for jj in range(2):
                        nc.tensor.matmul(
                            p_s[:, jj, :],
                            lhsT=ctxs_sb[:, :, ts(2 * g + jj, 128)],
                            rhs=qw_sb, start=True, stop=True, perf_mode=DR)
                    nc.scalar.activation(
                        E8[:, ds(2 * g, 2), :], p_s, AF.Exp,
                        bias=bias_exp, scale=1.0)

                # ---- ET via paired fp8 transposes + uint16 copies ----
                # ET[q, jqp, jch, kt, c, lane]: value lane 0, junk lane 1
                ET = bb.tile([128, 2, NCH, 2, 512, 2], F8, tag="ET")
                for jch in range(NCH):
                    for jqp in range(2):
                        p_tr = tr_pool.tile([128, 2, 512, 2], F8, tag="tr")
                        for kt in range(2):
                            jq = 2 * jqp + kt
                            for j4 in range(4):
                                jc = 4 * jch + j4
                                nc.tensor.transpose(
                                    p_tr[:, kt, ds(128 * j4, 128), 0:1],
                                    E8[:, jc, ts(jq, 128)], ident8)
                        nc.vector.tensor_copy(
                            ET[:, jqp, jch].bitcast(U16), p_tr.bitcast(U16))

                # ---- t-matmul (ones column -> S) + scales ----
                t2 = bb.tile([128, NQT, 128], F8, tag="t2")
                qryR = bb.tile([128, NQT, 128], F8, tag="qryR")
                recipA = bb.tile([128, NQT], F32, tag="recipA")
                for jq in range(NQT):
                    p_t = ab_pool.tile([128, 512], F32, tag="ab")
                    for i in range(8):
                        nc.tensor.matmul(
                            p_t[:, 0:129],
                            lhsT=E8[:, ds(2 * i, 2), ts(jq, 128)],
                            rhs=ctxc_sb[:, ds(2 * i, 2), 0:129],
                            start=(i == 0), stop=(i == 7), perf_mode=DR)
                    # ones column holds 1/SA, so S-col = S/SA and
                    # reciprocal directly yields recipA = SA / S
                    nc.vector.reciprocal(recipA[:, jq:jq + 1],
                                         p_t[:, 128:129])
                    nc.gpsimd.tensor_scalar_mul(
                        qryR[:, jq, :], qt_sb[:, jq, :],
                        recipA[:, jq:jq + 1])
                    nc.gpsimd.tensor_scalar(
                        out=t2[:, jq, :], in0=p_t[:, 0:128],
                        scalar1=recipA[:, jq:jq + 1],
                        scalar2=recipA[:, jq:jq + 1],
                        op0=MULT, op1=MULT)

                # ---- a/b2 matmuls + epilogue per c-chunk ----
                sec_a = stg.tile([128, C], BF16, tag="sec_a")
                sec_ca = stg.tile([128, C], BF16, tag="sec_ca")
                sec_cb = stg.tile([128, C], BF16, tag="sec_cb")
                for jch in range(NCH):
                    p_a = ab_pool.tile([128, 512], F32, tag="ab")
                    p_b = ab_pool.tile([128, 512], F32, tag="ab")
                    for jqp in range(2):
                        rhs = ET[:, jqp, jch, :, :, 0:1]
                        nc.tensor.matmul(
                            p_a, lhsT=qryR[:, ds(2 * jqp, 2), :], rhs=rhs,
                            start=(jqp == 0), stop=(jqp == 1), perf_mode=DR)
                    for jqp in range(2):
                        rhs = ET[:, jqp, jch, :, :, 0:1]
                        nc.tensor.matmul(
                            p_b, lhsT=t2[:, ds(2 * jqp, 2), :], rhs=rhs,
                            start=(jqp == 0), stop=(jqp == 1), perf_mode=DR)
                    sl = ts(jch, 512)
                    # section a: descale by 1/SA (ACT helps on the tail batch)
                    if b == BPC - 1:
                        nc.scalar.mul(sec_a[:, sl], p_a, 1.0 / SA)
                    else:
                        nc.vector.tensor_scalar_mul(sec_a[:, sl], p_a, 1.0 / SA)
                    # section ctx*a = (p_a / SA) * ctx   (fused descale)
                    eng_ca = nc.vector if b == 0 else nc.gpsimd
                    eng_ca.scalar_tensor_tensor(
                        out=sec_ca[:, sl], in0=p_a, scalar=1.0 / SA,
                        in1=ctx8_sb[:, sl], op0=MULT, op1=MULT)
                    # section ctx*b2 = (p_b / SA^2) * ctx
                    eng_cb = nc.gpsimd if b == 0 else nc.vector
                    eng_cb.scalar_tensor_tensor(
                        out=sec_cb[:, sl], in0=p_b, scalar=1.0 / SA ** 2,
                        in1=ctx8_sb[:, sl], op0=MULT, op1=MULT)
                    if jch == 1:
                        for sec, tl_ in ((0, sec_a), (1, sec_ca), (2, sec_cb)):
                            nc.sync.dma_start(out=out_ext[b, sec, :, 0:1024],
                                              in_=tl_[:, 0:1024])
                for sec, tl_ in ((0, sec_a), (1, sec_ca), (2, sec_cb)):
                    nc.sync.dma_start(out=out_ext[b, sec, :, 1024:2048],
                                      in_=tl_[:, 1024:2048])
    _split_multi_waits(nc)
    return nc


_NC = None


def _prep_inputs(context, query, w):
    """Host-side sharding + fp8 packing into the two byte tensors."""
    f32 = np.float32
    context = np.ascontiguousarray(context, dtype=f32)
    query = np.ascontiguousarray(query, dtype=f32)
    w = np.ascontiguousarray(w, dtype=f32)
    wc, wcq = w[D:2 * D], w[2 * D:3 * D]

    ctx_s = np.ascontiguousarray(
        context.reshape(B, 2, 64, C).transpose(0, 2, 1, 3)).astype(F8NP)
    qw = np.ascontiguousarray(
        (query * wcq[None, :, None] + wc[None, :, None])
        .reshape(B, 2, 64, Q).transpose(0, 2, 1, 3)).astype(F8NP)
    pk64 = np.concatenate([
        ctx_s.reshape(B, 64, 2 * C).view(np.uint8),
        qw.reshape(B, 64, 2 * Q).view(np.uint8)], axis=2)

    # ctxc[b, p, j, 0:128] = ctx[b, :, 128j+p]; col 128 = 1/SA (S column)
    ctxc = np.zeros((B, 128, NCT, 132), dtype=F8NP)
    ctxc[:, :, :, 0:128] = (context.transpose(0, 2, 1)
                            .reshape(B, NCT, 128, D)
                            .transpose(0, 2, 1, 3)).astype(F8NP)
    ctxc[:, :, :, 128] = np.float32(1.0 / SA).astype(F8NP)
    qt = np.ascontiguousarray(
        query.transpose(0, 2, 1).reshape(B, NQT, 128, D)
        .transpose(0, 2, 1, 3)).astype(F8NP)
    ctx8 = context.astype(F8NP)
    pk128 = np.concatenate([
        ctxc.reshape(B, 128, NCT * 132).view(np.uint8),
        qt.reshape(B, 128, NQT * 128).view(np.uint8),
        ctx8.view(np.uint8)], axis=2)

    in_maps = []
    for i in range(NCORES):
        sl = slice(i * BPC, (i + 1) * BPC)
        in_maps.append({"pk64": pk64[sl], "pk128": pk128[sl]})
    return context, in_maps


def kernel(context: np.ndarray, query: np.ndarray, w: np.ndarray,
           **extra) -> np.ndarray:
    global _NC
    if _NC is None:
        _NC = build_kernel()
    context, in_maps = _prep_inputs(context, query, w)
    res = run_bass_kernel_spmd(_NC, in_maps, core_ids=list(range(NCORES)))
    dev = np.concatenate([r["out"] for r in res.results], axis=0)  # [B,3,128,C]
    out = np.empty((B, 4 * D, C), dtype=np.float32)
    out[:, 0:D, :] = context
    out[:, D:4 * D, :] = dev.astype(np.float32).reshape(B, 3 * D, C)
    return out


if __name__ == "__main__":
    rng = np.random.default_rng(0)
    out = kernel(
        context=rng.standard_normal((B, D, C), dtype=np.float32),
        query=rng.standard_normal((B, D, Q), dtype=np.float32),
        w=(rng.random(3 * D, dtype=np.float32) - 0.5) * 2 / np.sqrt(D),
    )
    print(out.shape, out.dtype)
